# revision 8
# baseline (speedup 1.0000x reference)
"""GQA attention kernel for 8 trn2 NeuronCores (v2).

Sharding: core c in 0..7 -> batch b = c//4, KV group g = c%4 (4 Q heads,
1 KV head per core). Host sums the 4 partial outputs per batch.

v2 design notes:
- All projection/attention matmuls run in bf16 (1 cyc/row on PE, same as
  fp32r, but half the DMA/SBUF traffic and 2x DVE throughput).
- Softmax: S^T tiles (k on partitions) -> one wide [128,1024] exp per two
  k-tiles on the Act engine; sum over k via a DVE add-chain plus a GpSimd
  partition_all_reduce; normalization via DVE reciprocal+mult. The PE does
  zero softmax work.
- RoPE roll-by-1 over head dim = partition rotate, done with SBUF->SBUF
  DMAs instead of PE shift-matmuls.
- Out-projection (Wo) matmuls are interleaved into the attention stream as
  PE filler; results DMA straight from PSUM to DRAM except the final tile
  group, which stages through SBUF.
"""
import sys
sys.path.insert(0, "/opt/trn_rl_repo")
import math
import numpy as np

B, L, D = 2, 2048, 2048
H, HKV, HD = 16, 4, 128
BASE = 10000.0
NT = L // 128      # 16 seq tiles of 128
NCH = L // 512     # 4 seq chunks of 512
NH = H // HKV      # 4 heads per core
NG = 8             # 8 wide S/exp groups per combo (2 k-tiles each)
SCALE = 1.0 / math.sqrt(HD)

_compiled = None


def _build():
    from concourse import bacc, tile, mybir
    import concourse.bass_isa as bass_isa

    f32, f32r, bf16 = mybir.dt.float32, mybir.dt.float32r, mybir.dt.bfloat16
    Exp = mybir.ActivationFunctionType.Exp
    Copy = mybir.ActivationFunctionType.Copy
    mult, add = mybir.AluOpType.mult, mybir.AluOpType.add

    nc = bacc.Bacc("TRN2", target_bir_lowering=False, debug=False,
                   enable_asserts=True, num_devices=8)

    # host-relayouted inputs (see _host_inputs)
    xh_d = nc.dram_tensor("xh", [128, NCH * NT * 512], bf16, kind="ExternalInput")
    wq_d = nc.dram_tensor("wq", [128, NT * 512], bf16, kind="ExternalInput")
    wk_d = nc.dram_tensor("wk", [128, NT * 128], bf16, kind="ExternalInput")
    wv_d = nc.dram_tensor("wv", [128, NT * 128], bf16, kind="ExternalInput")
    wo_d = nc.dram_tensor("wo", [NH * HD, D], bf16, kind="ExternalInput")
    cos_d = nc.dram_tensor("cosT", [HD, L], bf16, kind="ExternalInput")
    idn_d = nc.dram_tensor("ident", [128, 128], f32, kind="ExternalInput")
    sin_d = nc.dram_tensor("sinT", [HD, L], bf16, kind="ExternalInput")
    y_d = nc.dram_tensor("y", [L, D], f32, kind="ExternalOutput")

    with tile.TileContext(nc, pool_alloc_mode="queue") as tc, \
         nc.allow_low_precision(reason="bf16 matmul path; softmax sums in "
                                "bf16 with fp32 cross-partition reduce"):
        with tc.tile_pool(name="persist", bufs=1) as pp:
            idn = pp.tile([128, 128], f32, tag="idn", name="idn")
            qt = [[pp.tile([HD, 512], bf16, tag=f"qt{h}_{n}", name=f"qt{h}_{n}")
                   for n in range(NCH)] for h in range(NH)]
            kt = [pp.tile([HD, 512], bf16, tag=f"kt{n}", name=f"kt{n}")
                  for n in range(NCH)]
            vn = [pp.tile([128, HD], bf16, tag=f"vn{t}", name=f"vn{t}")
                  for t in range(NT)]
            ot = [[pp.tile([HD, 512], bf16, tag=f"ot{h}_{q}", name=f"ot{h}_{q}")
                   for q in range(NCH)] for h in range(NH)]
            wo = [pp.tile([HD, L], bf16, tag=f"wo{h}", name=f"wo{h}")
                  for h in range(NH)]
            espre = {(h, g): pp.tile([128, 1024], bf16, tag=f"espre{h}_{g}",
                                     name=f"espre{h}_{g}")
                     for h in range(2) for g in range(6)}

            # =========== Phase A: projections + RoPE + V transpose =========
            with tc.tile_pool(name="ax", bufs=3) as ax, \
                 tc.tile_pool(name="aw", bufs=1) as aw, \
                 tc.tile_pool(name="arope", bufs=1) as ar, \
                 tc.tile_pool(name="apsum", bufs=1, space="PSUM") as aps:
                cs = aw.tile([HD, L], bf16, tag="cs", name="cs")
                sn = aw.tile([HD, L], bf16, tag="sn", name="sn")
                wq_sb = aw.tile([128, NT * 512], bf16, tag="wqsb", name="wqsb")
                wk_sb = aw.tile([128, NT * 128], bf16, tag="wksb", name="wksb")
                wv_sb = aw.tile([128, NT * 128], bf16, tag="wvsb", name="wvsb")
                vf = [None] * NCH

                xcb = {}   # (n, g) -> batched x tile covering c in [4g, 4g+4)

                def prefetch_xcb(n, g):
                    t = ax.tile([128, 2048], bf16, tag=f"xcb{g}",
                                name=f"xcb{g}", bufs=1)
                    nc.sync.dma_start(
                        t[:], xh_d[:, (n * NT + 4 * g) * 512:
                                   (n * NT + 4 * g + 4) * 512])
                    xcb[(n, g)] = t

                # chunk-0 x tiles interleave with the weight stream below
                xc0 = ax.tile([128, 512], bf16, tag="xc0", name="xc0", bufs=1)
                for n in range(NCH):
                    ps = [aps.tile([128, 512], f32, tag=f"pa{j}", name=f"pa{j}")
                          for j in range(6)]
                    for c in range(NT):
                        if n == 0:
                            # cold start, paced so DMA never outruns ~360GB/s:
                            # weights per-c early, everything else spread out.
                            # c=0 pieces are minimal so the PE starts ~2us in.
                            if c == 0:
                                # first-wave loads split across the three DMA
                                # issue paths (SP/Act via HWDGE, Pool via
                                # SWDGE) so HWDGE serialization doesn't gate
                                # the first matmul
                                nc.sync.dma_start(wq_sb[:, 0:512],
                                                  wq_d[:, 0:512])
                                nc.scalar.dma_start(xc0[:],
                                                    xh_d[:, 0:512])
                                nc.gpsimd.dma_start(wk_sb[:, 0:512],
                                                    wk_d[:, 0:512])
                                nc.gpsimd.dma_start(wv_sb[:, 0:512],
                                                    wv_d[:, 0:512])
                                prefetch_xcb(0, 0)
                            elif c < 8:
                                nc.sync.dma_start(
                                    wq_sb[:, c * 512:(c + 1) * 512],
                                    wq_d[:, c * 512:(c + 1) * 512])
                            elif c == 8:
                                nc.sync.dma_start(wq_sb[:, 8 * 512:NT * 512],
                                                  wq_d[:, 8 * 512:NT * 512])
                            if c == 1:
                                nc.sync.dma_start(wk_sb[:, 512:2048],
                                                  wk_d[:, 512:2048])
                                prefetch_xcb(0, 1)
                            if c == 2:
                                nc.sync.dma_start(wv_sb[:, 512:2048],
                                                  wv_d[:, 512:2048])
                            if c == 5:
                                prefetch_xcb(0, 2)
                            if c == 9:
                                prefetch_xcb(0, 3)
                            if c == 10:
                                nc.sync.dma_start(cs[:], cos_d[:])
                            if c == 11:
                                nc.sync.dma_start(sn[:], sin_d[:])
                                nc.sync.dma_start(idn[:], idn_d[:])
                        if n > 0 and 4 <= c < 8:
                            t = c - 4
                            nc.sync.dma_start_transpose(
                                vn[(n - 1) * 4 + t][:],
                                vf[n - 1][:, t * 128:(t + 1) * 128])
                        # combos (0,0)/(0,1) S+exp prework: the Act engine
                        # is idle during phase A, so the first two combos'
                        # softmax inputs are ready when phase B starts
                        gpre = {(1, 4): (0, 0), (1, 7): (0, 1),
                                (1, 10): (1, 0), (1, 13): (1, 1),
                                (2, 4): (0, 2), (2, 7): (0, 3),
                                (2, 10): (1, 2), (2, 13): (1, 3),
                                (3, 4): (0, 4), (3, 7): (0, 5),
                                (3, 10): (1, 4), (3, 13): (1, 5)}.get((n, c))
                        if gpre is not None:
                            hp, gp = gpre
                            psp = aps.tile([128, 1024], f32, tag="psp",
                                           name="psp")
                            for half in range(2):
                                k = 2 * gp + half
                                nc.tensor.matmul(
                                    psp[:, half * 512:(half + 1) * 512],
                                    kt[k // 4][:, (k % 4) * 128:(k % 4 + 1) * 128],
                                    qt[hp][0][:], start=True, stop=True)
                            nc.scalar.activation(espre[(hp, gp)][:], psp[:],
                                                 Exp, scale=SCALE)
                        if n == 0 and c == 0:
                            xc = xc0[:]
                        else:
                            xc = xcb[(n, c // 4)][:, (c % 4) * 512:
                                                  (c % 4 + 1) * 512]
                        # prefetch next chunk's batched x tiles
                        if n < NCH - 1 and c >= 12:
                            prefetch_xcb(n + 1, c - 12)
                        for j in range(NH):
                            nc.tensor.matmul(
                                ps[j][:],
                                wq_sb[:, c * 512 + j * 128:c * 512 + (j + 1) * 128],
                                xc, start=(c == 0), stop=(c == NT - 1))
                        nc.tensor.matmul(ps[4][:], wk_sb[:, c * 128:(c + 1) * 128],
                                         xc, start=(c == 0), stop=(c == NT - 1))
                        nc.tensor.matmul(ps[5][:], wv_sb[:, c * 128:(c + 1) * 128],
                                         xc, start=(c == 0), stop=(c == NT - 1))
                        if n == 2 and c >= 8 and c % 2 == 0:
                            # 1MB each, spread so x-tile prefetch never starves
                            h = (c - 8) // 2
                            nc.sync.dma_start(
                                wo[h][:], wo_d[h * HD:(h + 1) * HD, :])

                    # V: PSUM -> SBUF (bf16), then four xbar DMA
                    # transposes straight into the vn tiles — zero PE work.
                    last = n == NCH - 1
                    vf[n] = (pp.tile([128, 512], f32, tag="vflast",
                                     name="vflast") if last else
                             ar.tile([128, 512], bf16, tag=f"vf{n % 2}",
                                     name=f"vf{n % 2}"))
                    if last:
                        nc.scalar.activation(vf[n][:], ps[5][:], Copy)
                    else:
                        nc.vector.tensor_copy(vf[n][:], ps[5][:])

                    # RoPE. Non-last chunks: all five PSUM->SBUF copies on
                    # Act, ordered to free pa0 first. Last chunk: K on Act
                    # (kt[3] gates early S-groups) but Q planes on DVE so the
                    # Act queue reaches phase B's exps immediately.
                    csl = cs[:, n * 512:(n + 1) * 512]
                    ssl = sn[:, n * 512:(n + 1) * 512]
                    raw = ar.tile([128, 2560], bf16, tag="raw", name="raw",
                                  bufs=3)
                    prl = ar.tile([128, 2560], bf16, tag="prl", name="prl",
                                  bufs=2)
                    order = [4, 0, 1, 2, 3] if last else [0, 4, 1, 2, 3]
                    off = {}
                    for i, jj in enumerate(order):
                        off[jj] = i * 512
                        src, dst = ps[jj][:], raw[:, i * 512:(i + 1) * 512]
                        if last and jj != 4:
                            nc.vector.tensor_copy(dst, src)
                        else:
                            nc.scalar.activation(dst, src, Copy)
                        if last and i == 0:    # rotate K alone, immediately
                            nc.sync.dma_start(prl[1:128, 0:512],
                                              raw[0:127, 0:512])
                            nc.sync.dma_start(prl[0:1, 0:512],
                                              raw[127:128, 0:512])
                        elif not last and i == 1:
                            nc.sync.dma_start(prl[1:128, 0:1024],
                                              raw[0:127, 0:1024])
                            nc.sync.dma_start(prl[0:1, 0:1024],
                                              raw[127:128, 0:1024])
                    lo = 512 if last else 1024
                    nc.sync.dma_start(prl[1:128, lo:2560], raw[0:127, lo:2560])
                    nc.sync.dma_start(prl[0:1, lo:2560],
                                      raw[127:128, lo:2560])
                    for jj in order:
                        o = off[jj]
                        t1 = ar.tile([128, 512], bf16, tag="t1", name="t1",
                                     bufs=4)
                        nc.vector.tensor_tensor(t1[:], raw[:, o:o + 512], csl,
                                                mult)
                        t2 = ar.tile([128, 512], bf16, tag="t2", name="t2",
                                     bufs=4)
                        nc.vector.tensor_tensor(t2[:], prl[:, o:o + 512], ssl,
                                                mult)
                        dst = kt[n] if jj == 4 else qt[jj][n]
                        nc.vector.tensor_tensor(dst[:], t1[:], t2[:], add)

            # ====== Phase B: attention; Phase C: out-proj as PE filler =====
            with tc.tile_pool(name="bexp", bufs=1) as bx, \
                 tc.tile_pool(name="ysb", bufs=1) as yp, \
                 tc.tile_pool(name="bpsum", bufs=1, space="PSUM") as bps, \
                 tc.tile_pool(name="cpsum", bufs=1, space="PSUM") as cps:

                # pending out-projection work, emitted piecewise as PE filler
                pend = []

                def make_cwork(qb, split_store=False):
                    """Emission closures for out-proj of query-block qb:
                    per row-tile, 16 matmuls into PSUM (interleavable PE
                    filler), copied into an SBUF row (alternating DVE/Act so
                    copies never pace the PSUM banks), then DMA'd out. The
                    final block stores per-chunk so the last DMA is small."""
                    work = []
                    for ti in range(4):
                        qtile = qb * 4 + ti
                        box = {}

                        def mkrow(box=box):
                            box["ysb"] = yp.tile([128, L], f32, tag="ysbt",
                                                 name="ysbt", bufs=4)

                        for nn in range(NCH):
                            def mm(h, ti=ti, nn=nn, qb=qb, box=box):
                                if h == 0 and nn == 0:
                                    mkrow(box)
                                if h == 0:
                                    box["psy"] = cps.tile(
                                        [128, 512], f32, tag=f"psy{nn % 2}",
                                        name=f"psy{nn % 2}")
                                nc.tensor.matmul(
                                    box["psy"][:],
                                    ot[h][qb][:, ti * 128:(ti + 1) * 128],
                                    wo[h][:, nn * 512:(nn + 1) * 512],
                                    start=(h == 0), stop=(h == NH - 1))
                            for h in range(NH):
                                work.append(lambda h=h, mm=mm: mm(h))

                            def drain(nn=nn, box=box, qtile=qtile,
                                      split=split_store):
                                dst = box["ysb"][:, nn * 512:(nn + 1) * 512]
                                if split and nn % 2 == 1:
                                    nc.scalar.activation(dst, box["psy"][:],
                                                         Copy)
                                else:
                                    nc.vector.tensor_copy(dst, box["psy"][:])
                                if split:
                                    nc.sync.dma_start(
                                        y_d[qtile * 128:(qtile + 1) * 128,
                                            nn * 512:(nn + 1) * 512], dst)
                            work.append(drain)

                        if not split_store:
                            def store(qtile=qtile, box=box):
                                nc.sync.dma_start(
                                    y_d[qtile * 128:(qtile + 1) * 128, :],
                                    box["ysb"][:])
                            work.append(store)
                    return work

                def cfill(k):
                    for _ in range(k):
                        if pend:
                            pend.pop(0)()

                for qb in range(NCH):
                    if qb > 0:
                        pend.extend(make_cwork(qb - 1))
                    for h in range(NH):
                        es_map = {}
                        acc = None

                        def emit_S(g, h=h, qb=qb):
                            nonlocal acc
                            if qb == 0 and h < 2 and g < 6:
                                es = espre[(h, g)]   # computed during phase A
                            else:
                                pss = bps.tile([128, 1024], f32,
                                               tag=f"pss{g % 2}",
                                               name=f"pss{g % 2}")
                                for half in range(2):
                                    k = 2 * g + half
                                    nc.tensor.matmul(
                                        pss[:, half * 512:(half + 1) * 512],
                                        kt[k // 4][:, (k % 4) * 128:(k % 4 + 1) * 128],
                                        qt[h][qb][:], start=True, stop=True)
                                es = bx.tile([128, 1024], bf16, tag="es",
                                             name="es", bufs=8)
                                nc.scalar.activation(es[:], pss[:], Exp,
                                                     scale=SCALE)
                            es_map[g] = es
                            # running sum over groups on DVE
                            if acc is None:
                                acc = es
                            else:
                                nacc = bx.tile([128, 1024], bf16, tag="acc",
                                               name="acc", bufs=3)
                                nc.vector.tensor_tensor(nacc[:], acc[:], es[:],
                                                        add)
                                acc = nacc

                        pso_box = {}

                        def emit_PV(g, h=h, qb=qb):
                            if g == 0:
                                pso_box["pso"] = bps.tile(
                                    [HD, 512], f32, tag=f"pso{(qb * NH + h) % 2}",
                                    name=f"pso{(qb * NH + h) % 2}")
                            pso = pso_box["pso"]
                            for half in range(2):
                                k = 2 * g + half
                                nc.tensor.matmul(pso[:], vn[k][:],
                                                 es_map[g][:, half * 512:(half + 1) * 512],
                                                 start=(k == 0), stop=(k == NT - 1))

                        if qb == 0 and h < 2:
                            # most groups were pre-computed in phase A: do the
                            # two live S-groups first, then stream the PVs
                            emit_S(6)
                            emit_S(7)
                            if h == 0:
                                # last chunk's V transposes on the PE, staged
                                # in the (still idle) out-proj PSUM banks
                                pv0 = cps.tile([128, 512], f32, tag="psy0",
                                               name="psy0")
                                pv1 = cps.tile([128, 512], f32, tag="psy1",
                                               name="psy1")
                                for t in range(4):
                                    pvt = [pv0, pv1][t % 2][
                                        :, (t // 2) * 128:(t // 2 + 1) * 128]
                                    nc.tensor.transpose(
                                        pvt,
                                        vf[NCH - 1][:, t * 128:(t + 1) * 128],
                                        idn[:])
                                    nc.vector.tensor_copy(
                                        vn[(NCH - 1) * 4 + t][:], pvt)
                            for g in range(6):
                                emit_S(g)    # bookkeeping only (chain adds)
                            for g in range(NG):
                                emit_PV(g)
                        else:
                            emit_S(0)
                            emit_S(1)
                            for g in range(2, NG):
                                cfill(3 if len(pend) > 60 else (2 if len(pend) > 30 else 1))
                                emit_PV(g - 2)
                                emit_S(g)
                            cfill(1)
                            emit_PV(NG - 2)
                            cfill(1)
                            emit_PV(NG - 1)

                        # sumexp finalize: fold halves, cross-partition
                        # all-reduce on GpSimd, reciprocal, normalize.
                        sh = bx.tile([128, 512], bf16, tag="sh", name="sh",
                                     bufs=3)
                        nc.vector.tensor_tensor(sh[:], acc[:, 0:512],
                                                acc[:, 512:1024], add)
                        sums = bx.tile([128, 512], f32, tag="sums", name="sums",
                                       bufs=3)
                        nc.gpsimd.partition_all_reduce(
                            sums[:], sh[:], channels=128,
                            reduce_op=bass_isa.ReduceOp.add)
                        rec = bx.tile([128, 512], f32, tag="rec", name="rec",
                                      bufs=3)
                        nc.vector.reciprocal(rec[:], sums[:])
                        nc.vector.tensor_tensor(ot[h][qb][:], pso_box["pso"][:],
                                                rec[:], mult)

                    # drain whatever filler remains before the next qb
                    cfill(len(pend))

                # ---- tail: out-proj for the last query block
                pend.extend(make_cwork(NCH - 1, split_store=True))
                cfill(len(pend))

    nc.compile()
    return nc


def _host_inputs(x, Wq, Wk, Wv, Wo):
    import ml_dtypes
    bf16 = ml_dtypes.bfloat16

    inv = 1.0 / (BASE ** (np.arange(0, HD, 2, dtype=np.float32) / HD))
    pos = np.arange(L, dtype=np.float32)
    fr = pos[:, None] * inv[None, :]
    emb = np.concatenate([fr, fr], axis=1)            # [L, HD]
    cosT = np.ascontiguousarray(np.cos(emb).T).astype(bf16)   # [HD, L]
    sinT = np.ascontiguousarray(np.sin(emb).T).astype(bf16)
    idn = np.eye(128, dtype=np.float32)

    # x relayout: xh[p, (n*16+c)*512 + l] = x[b][n*512+l, c*128+p]
    xh = [np.ascontiguousarray(
        x[b].T.reshape(NT, 128, NCH, 512).transpose(1, 2, 0, 3)
        .reshape(128, NCH * NT * 512)).astype(bf16) for b in range(B)]

    maps = []
    for core in range(8):
        b, g = core // 4, core % 4
        WqS = Wq[:, g * NH * HD:(g + 1) * NH * HD]    # [D, 512]
        WkS = Wk[:, g * HD:(g + 1) * HD]              # [D, 128]
        WvS = Wv[:, g * HD:(g + 1) * HD]
        wq_h = np.ascontiguousarray(
            WqS.reshape(NT, 128, 512).transpose(1, 0, 2)
            .reshape(128, NT * 512)).astype(bf16)
        wk_h = np.ascontiguousarray(
            WkS.reshape(NT, 128, 128).transpose(1, 0, 2)
            .reshape(128, NT * 128)).astype(bf16)
        wv_h = np.ascontiguousarray(
            WvS.reshape(NT, 128, 128).transpose(1, 0, 2)
            .reshape(128, NT * 128)).astype(bf16)
        maps.append({
            "xh": xh[b],
            "wq": wq_h, "wk": wk_h, "wv": wv_h,
            "wo": np.ascontiguousarray(
                Wo[g * NH * HD:(g + 1) * NH * HD, :]).astype(bf16),
            "cosT": cosT, "sinT": sinT, "ident": idn,
        })
    return maps


def _run(inputs, trace=False):
    global _compiled
    from concourse.bass_utils import run_bass_kernel_spmd
    if _compiled is None:
        _compiled = _build()
    maps = _host_inputs(inputs["x"], inputs["Wq"], inputs["Wk"],
                        inputs["Wv"], inputs["Wo"])
    res = run_bass_kernel_spmd(_compiled, maps, list(range(8)), trace=trace)
    y = np.empty((B, L, D), np.float32)
    for b in range(B):
        y[b] = res.results[b * 4]["y"]
        for g in range(1, 4):
            y[b] += res.results[b * 4 + g]["y"]
    return y, res


def kernel(**inputs):
    x = np.asarray(inputs["x"], np.float32)
    y, _ = _run({"x": x,
                 "Wq": np.asarray(inputs["Wq"], np.float32),
                 "Wk": np.asarray(inputs["Wk"], np.float32),
                 "Wv": np.asarray(inputs["Wv"], np.float32),
                 "Wo": np.asarray(inputs["Wo"], np.float32)})
    return y


# revision 9
# speedup vs baseline: 1.0002x; 1.0002x over previous
"""GQA attention kernel for 8 trn2 NeuronCores (v2).

Sharding: core c in 0..7 -> batch b = c//4, KV group g = c%4 (4 Q heads,
1 KV head per core). Host sums the 4 partial outputs per batch.

v2 design notes:
- All projection/attention matmuls run in bf16 (1 cyc/row on PE, same as
  fp32r, but half the DMA/SBUF traffic and 2x DVE throughput).
- Softmax: S^T tiles (k on partitions) -> one wide [128,1024] exp per two
  k-tiles on the Act engine; sum over k via a DVE add-chain plus a GpSimd
  partition_all_reduce; normalization via DVE reciprocal+mult. The PE does
  zero softmax work.
- RoPE roll-by-1 over head dim = partition rotate, done with SBUF->SBUF
  DMAs instead of PE shift-matmuls.
- Out-projection (Wo) matmuls are interleaved into the attention stream as
  PE filler; results DMA straight from PSUM to DRAM except the final tile
  group, which stages through SBUF.
"""
import sys
sys.path.insert(0, "/opt/trn_rl_repo")
import math
import numpy as np

B, L, D = 2, 2048, 2048
H, HKV, HD = 16, 4, 128
BASE = 10000.0
NT = L // 128      # 16 seq tiles of 128
NCH = L // 512     # 4 seq chunks of 512
NH = H // HKV      # 4 heads per core
NG = 8             # 8 wide S/exp groups per combo (2 k-tiles each)
SCALE = 1.0 / math.sqrt(HD)

_compiled = None


def _build():
    from concourse import bacc, tile, mybir
    import concourse.bass_isa as bass_isa

    f32, f32r, bf16 = mybir.dt.float32, mybir.dt.float32r, mybir.dt.bfloat16
    Exp = mybir.ActivationFunctionType.Exp
    Copy = mybir.ActivationFunctionType.Copy
    mult, add = mybir.AluOpType.mult, mybir.AluOpType.add

    nc = bacc.Bacc("TRN2", target_bir_lowering=False, debug=False,
                   enable_asserts=True, num_devices=8)

    # host-relayouted inputs (see _host_inputs)
    xh_d = nc.dram_tensor("xh", [128, NCH * NT * 512], bf16, kind="ExternalInput")
    wq_d = nc.dram_tensor("wq", [128, NT * 512], bf16, kind="ExternalInput")
    wk_d = nc.dram_tensor("wk", [128, NT * 128], bf16, kind="ExternalInput")
    wv_d = nc.dram_tensor("wv", [128, NT * 128], bf16, kind="ExternalInput")
    wo_d = nc.dram_tensor("wo", [NH * HD, D], bf16, kind="ExternalInput")
    cos_d = nc.dram_tensor("cosT", [HD, L], bf16, kind="ExternalInput")
    idn_d = nc.dram_tensor("ident", [128, 128], f32, kind="ExternalInput")
    sin_d = nc.dram_tensor("sinT", [HD, L], bf16, kind="ExternalInput")
    y_d = nc.dram_tensor("y", [L, D], f32, kind="ExternalOutput")

    with tile.TileContext(nc, pool_alloc_mode="queue") as tc, \
         nc.allow_low_precision(reason="bf16 matmul path; softmax sums in "
                                "bf16 with fp32 cross-partition reduce"):
        with tc.tile_pool(name="persist", bufs=1) as pp:
            idn = pp.tile([128, 128], f32, tag="idn", name="idn")
            qt = [[pp.tile([HD, 512], bf16, tag=f"qt{h}_{n}", name=f"qt{h}_{n}")
                   for n in range(NCH)] for h in range(NH)]
            kt = [pp.tile([HD, 512], bf16, tag=f"kt{n}", name=f"kt{n}")
                  for n in range(NCH)]
            vn = [pp.tile([128, HD], bf16, tag=f"vn{t}", name=f"vn{t}")
                  for t in range(NT)]
            ot = [[pp.tile([HD, 512], bf16, tag=f"ot{h}_{q}", name=f"ot{h}_{q}")
                   for q in range(NCH)] for h in range(NH)]
            wo = [pp.tile([HD, L], bf16, tag=f"wo{h}", name=f"wo{h}")
                  for h in range(NH)]
            espre = {(h, g): pp.tile([128, 1024], bf16, tag=f"espre{h}_{g}",
                                     name=f"espre{h}_{g}")
                     for h in range(2) for g in range(6)}

            # =========== Phase A: projections + RoPE + V transpose =========
            with tc.tile_pool(name="ax", bufs=3) as ax, \
                 tc.tile_pool(name="aw", bufs=1) as aw, \
                 tc.tile_pool(name="arope", bufs=1) as ar, \
                 tc.tile_pool(name="apsum", bufs=1, space="PSUM") as aps:
                cs = aw.tile([HD, L], bf16, tag="cs", name="cs")
                sn = aw.tile([HD, L], bf16, tag="sn", name="sn")
                wq_sb = aw.tile([128, NT * 512], bf16, tag="wqsb", name="wqsb")
                wk_sb = aw.tile([128, NT * 128], bf16, tag="wksb", name="wksb")
                wv_sb = aw.tile([128, NT * 128], bf16, tag="wvsb", name="wvsb")
                vf = [None] * NCH

                xcb = {}   # (n, g) -> batched x tile covering c in [4g, 4g+4)

                def prefetch_xcb(n, g):
                    t = ax.tile([128, 2048], bf16, tag=f"xcb{g}",
                                name=f"xcb{g}", bufs=1)
                    nc.sync.dma_start(
                        t[:], xh_d[:, (n * NT + 4 * g) * 512:
                                   (n * NT + 4 * g + 4) * 512])
                    xcb[(n, g)] = t

                # chunk-0 x tiles interleave with the weight stream below
                xc0 = ax.tile([128, 512], bf16, tag="xc0", name="xc0", bufs=1)
                for n in range(NCH):
                    ps = [aps.tile([128, 512], f32, tag=f"pa{j}", name=f"pa{j}")
                          for j in range(6)]
                    for c in range(NT):
                        if n == 0:
                            # cold start, paced so DMA never outruns ~360GB/s:
                            # weights per-c early, everything else spread out.
                            # c=0 pieces are minimal so the PE starts ~2us in.
                            if c == 0:
                                # first-wave loads split across the three DMA
                                # issue paths (SP/Act via HWDGE, Pool via
                                # SWDGE) so HWDGE serialization doesn't gate
                                # the first matmul
                                nc.sync.dma_start(wq_sb[:, 0:512],
                                                  wq_d[:, 0:512])
                                nc.scalar.dma_start(xc0[:],
                                                    xh_d[:, 0:512])
                                nc.gpsimd.dma_start(wk_sb[:, 0:512],
                                                    wk_d[:, 0:512])
                                nc.gpsimd.dma_start(wv_sb[:, 0:512],
                                                    wv_d[:, 0:512])
                                prefetch_xcb(0, 0)
                            elif c < 8:
                                nc.sync.dma_start(
                                    wq_sb[:, c * 512:(c + 1) * 512],
                                    wq_d[:, c * 512:(c + 1) * 512])
                            elif c == 8:
                                nc.sync.dma_start(wq_sb[:, 8 * 512:NT * 512],
                                                  wq_d[:, 8 * 512:NT * 512])
                            if c == 1:
                                nc.sync.dma_start(wk_sb[:, 512:2048],
                                                  wk_d[:, 512:2048])
                                prefetch_xcb(0, 1)
                            if c == 2:
                                nc.sync.dma_start(wv_sb[:, 512:2048],
                                                  wv_d[:, 512:2048])
                            if c == 5:
                                prefetch_xcb(0, 2)
                            if c == 9:
                                prefetch_xcb(0, 3)
                            if c == 10:
                                nc.sync.dma_start(cs[:], cos_d[:])
                            if c == 11:
                                nc.sync.dma_start(sn[:], sin_d[:])
                                nc.sync.dma_start(idn[:], idn_d[:])
                        if n > 0 and 4 <= c < 8:
                            t = c - 4
                            nc.sync.dma_start_transpose(
                                vn[(n - 1) * 4 + t][:],
                                vf[n - 1][:, t * 128:(t + 1) * 128])
                        # combos (0,0)/(0,1) S+exp prework: the Act engine
                        # is idle during phase A, so the first two combos'
                        # softmax inputs are ready when phase B starts
                        gpre = {(1, 4): (0, 0), (1, 7): (0, 1),
                                (1, 10): (1, 0), (1, 13): (1, 1),
                                (2, 4): (0, 2), (2, 7): (0, 3),
                                (2, 10): (1, 2), (2, 13): (1, 3),
                                (3, 4): (0, 4), (3, 7): (0, 5),
                                (3, 10): (1, 4), (3, 13): (1, 5)}.get((n, c))
                        if gpre is not None:
                            hp, gp = gpre
                            psp = aps.tile([128, 1024], f32, tag="psp",
                                           name="psp")
                            for half in range(2):
                                k = 2 * gp + half
                                nc.tensor.matmul(
                                    psp[:, half * 512:(half + 1) * 512],
                                    kt[k // 4][:, (k % 4) * 128:(k % 4 + 1) * 128],
                                    qt[hp][0][:], start=True, stop=True)
                            nc.scalar.activation(espre[(hp, gp)][:], psp[:],
                                                 Exp, scale=SCALE)
                        if n == 0 and c == 0:
                            xc = xc0[:]
                        else:
                            xc = xcb[(n, c // 4)][:, (c % 4) * 512:
                                                  (c % 4 + 1) * 512]
                        # prefetch next chunk's batched x tiles
                        if n < NCH - 1 and c >= 12:
                            prefetch_xcb(n + 1, c - 12)
                        for j in range(NH):
                            nc.tensor.matmul(
                                ps[j][:],
                                wq_sb[:, c * 512 + j * 128:c * 512 + (j + 1) * 128],
                                xc, start=(c == 0), stop=(c == NT - 1))
                        nc.tensor.matmul(ps[4][:], wk_sb[:, c * 128:(c + 1) * 128],
                                         xc, start=(c == 0), stop=(c == NT - 1))
                        nc.tensor.matmul(ps[5][:], wv_sb[:, c * 128:(c + 1) * 128],
                                         xc, start=(c == 0), stop=(c == NT - 1))
                        if n == 2 and c >= 8 and c % 2 == 0:
                            # 1MB each, spread so x-tile prefetch never starves
                            h = (c - 8) // 2
                            nc.sync.dma_start(
                                wo[h][:], wo_d[h * HD:(h + 1) * HD, :])

                    # V: PSUM -> SBUF (bf16), then four xbar DMA
                    # transposes straight into the vn tiles — zero PE work.
                    last = n == NCH - 1
                    vf[n] = (pp.tile([128, 512], f32, tag="vflast",
                                     name="vflast") if last else
                             ar.tile([128, 512], bf16, tag=f"vf{n % 2}",
                                     name=f"vf{n % 2}"))
                    if last:
                        nc.scalar.activation(vf[n][:], ps[5][:], Copy)
                    else:
                        nc.vector.tensor_copy(vf[n][:], ps[5][:])

                    # RoPE. Non-last chunks: all five PSUM->SBUF copies on
                    # Act, ordered to free pa0 first. Last chunk: K on Act
                    # (kt[3] gates early S-groups) but Q planes on DVE so the
                    # Act queue reaches phase B's exps immediately.
                    csl = cs[:, n * 512:(n + 1) * 512]
                    ssl = sn[:, n * 512:(n + 1) * 512]
                    raw = ar.tile([128, 2560], bf16, tag="raw", name="raw",
                                  bufs=3)
                    prl = ar.tile([128, 2560], bf16, tag="prl", name="prl",
                                  bufs=2)
                    order = [4, 0, 1, 2, 3] if last else [0, 4, 1, 2, 3]
                    off = {}
                    for i, jj in enumerate(order):
                        off[jj] = i * 512
                        src, dst = ps[jj][:], raw[:, i * 512:(i + 1) * 512]
                        if last and jj != 4:
                            nc.vector.tensor_copy(dst, src)
                        else:
                            nc.scalar.activation(dst, src, Copy)
                        if last and i == 0:    # rotate K alone, immediately
                            nc.sync.dma_start(prl[1:128, 0:512],
                                              raw[0:127, 0:512])
                            nc.sync.dma_start(prl[0:1, 0:512],
                                              raw[127:128, 0:512])
                        elif not last and i == 1:
                            nc.sync.dma_start(prl[1:128, 0:1024],
                                              raw[0:127, 0:1024])
                            nc.sync.dma_start(prl[0:1, 0:1024],
                                              raw[127:128, 0:1024])
                    lo = 512 if last else 1024
                    nc.sync.dma_start(prl[1:128, lo:2560], raw[0:127, lo:2560])
                    nc.sync.dma_start(prl[0:1, lo:2560],
                                      raw[127:128, lo:2560])
                    for jj in order:
                        o = off[jj]
                        t1 = ar.tile([128, 512], bf16, tag="t1", name="t1",
                                     bufs=4)
                        nc.vector.tensor_tensor(t1[:], raw[:, o:o + 512], csl,
                                                mult)
                        t2 = ar.tile([128, 512], bf16, tag="t2", name="t2",
                                     bufs=4)
                        nc.vector.tensor_tensor(t2[:], prl[:, o:o + 512], ssl,
                                                mult)
                        dst = kt[n] if jj == 4 else qt[jj][n]
                        nc.vector.tensor_tensor(dst[:], t1[:], t2[:], add)

            # ====== Phase B: attention; Phase C: out-proj as PE filler =====
            with tc.tile_pool(name="bexp", bufs=1) as bx, \
                 tc.tile_pool(name="ysb", bufs=1) as yp, \
                 tc.tile_pool(name="bpsum", bufs=1, space="PSUM") as bps, \
                 tc.tile_pool(name="cpsum", bufs=1, space="PSUM") as cps:

                # pending out-projection work, emitted piecewise as PE filler
                pend = []

                def make_cwork(qb, split_store=False):
                    """Emission closures for out-proj of query-block qb:
                    per row-tile, 16 matmuls into PSUM (interleavable PE
                    filler), copied into an SBUF row (alternating DVE/Act so
                    copies never pace the PSUM banks), then DMA'd out. The
                    final block stores per-chunk so the last DMA is small."""
                    work = []
                    for ti in range(4):
                        qtile = qb * 4 + ti
                        box = {}

                        def mkrow(box=box):
                            box["ysb"] = yp.tile([128, L], f32, tag="ysbt",
                                                 name="ysbt", bufs=4)

                        for nn in range(NCH):
                            def mm(h, ti=ti, nn=nn, qb=qb, box=box):
                                if h == 0 and nn == 0:
                                    mkrow(box)
                                if h == 0:
                                    box["psy"] = cps.tile(
                                        [128, 512], f32, tag=f"psy{nn % 2}",
                                        name=f"psy{nn % 2}")
                                nc.tensor.matmul(
                                    box["psy"][:],
                                    ot[h][qb][:, ti * 128:(ti + 1) * 128],
                                    wo[h][:, nn * 512:(nn + 1) * 512],
                                    start=(h == 0), stop=(h == NH - 1))
                            for h in range(NH):
                                work.append(lambda h=h, mm=mm: mm(h))

                            def drain(nn=nn, box=box, qtile=qtile,
                                      split=split_store):
                                dst = box["ysb"][:, nn * 512:(nn + 1) * 512]
                                if split and nn % 2 == 1:
                                    nc.scalar.activation(dst, box["psy"][:],
                                                         Copy)
                                else:
                                    nc.vector.tensor_copy(dst, box["psy"][:])
                                if split:
                                    nc.sync.dma_start(
                                        y_d[qtile * 128:(qtile + 1) * 128,
                                            nn * 512:(nn + 1) * 512], dst)
                            work.append(drain)

                        if not split_store:
                            def store(qtile=qtile, box=box):
                                nc.sync.dma_start(
                                    y_d[qtile * 128:(qtile + 1) * 128, :],
                                    box["ysb"][:])
                            work.append(store)
                    return work

                def cfill(k):
                    for _ in range(k):
                        if pend:
                            pend.pop(0)()

                for qb in range(NCH):
                    if qb > 0:
                        pend.extend(make_cwork(qb - 1))
                    for h in range(NH):
                        es_map = {}
                        acc = None

                        def emit_S(g, h=h, qb=qb):
                            nonlocal acc
                            if qb == 0 and h < 2 and g < 6:
                                es = espre[(h, g)]   # computed during phase A
                            else:
                                pss = bps.tile([128, 1024], f32,
                                               tag=f"pss{g % 2}",
                                               name=f"pss{g % 2}")
                                for half in range(2):
                                    k = 2 * g + half
                                    nc.tensor.matmul(
                                        pss[:, half * 512:(half + 1) * 512],
                                        kt[k // 4][:, (k % 4) * 128:(k % 4 + 1) * 128],
                                        qt[h][qb][:], start=True, stop=True)
                                es = bx.tile([128, 1024], bf16, tag="es",
                                             name="es", bufs=7)
                                nc.scalar.activation(es[:], pss[:], Exp,
                                                     scale=SCALE)
                            es_map[g] = es
                            # running sum over groups on DVE
                            if acc is None:
                                acc = es
                            else:
                                nacc = bx.tile([128, 1024], bf16, tag="acc",
                                               name="acc", bufs=3)
                                nc.vector.tensor_tensor(nacc[:], acc[:], es[:],
                                                        add)
                                acc = nacc

                        pso_box = {}

                        def emit_PV(g, h=h, qb=qb):
                            if g == 0:
                                pso_box["pso"] = bps.tile(
                                    [HD, 512], f32, tag=f"pso{(qb * NH + h) % 2}",
                                    name=f"pso{(qb * NH + h) % 2}")
                            pso = pso_box["pso"]
                            for half in range(2):
                                k = 2 * g + half
                                nc.tensor.matmul(pso[:], vn[k][:],
                                                 es_map[g][:, half * 512:(half + 1) * 512],
                                                 start=(k == 0), stop=(k == NT - 1))

                        if qb == 0 and h < 2:
                            # most groups were pre-computed in phase A: do the
                            # two live S-groups first, then stream the PVs
                            emit_S(6)
                            emit_S(7)
                            if h == 0:
                                # last chunk's V transposes on the PE, staged
                                # in the (still idle) out-proj PSUM banks
                                pv0 = cps.tile([128, 512], f32, tag="psy0",
                                               name="psy0")
                                pv1 = cps.tile([128, 512], f32, tag="psy1",
                                               name="psy1")
                                for t in range(4):
                                    pvt = [pv0, pv1][t % 2][
                                        :, (t // 2) * 128:(t // 2 + 1) * 128]
                                    nc.tensor.transpose(
                                        pvt,
                                        vf[NCH - 1][:, t * 128:(t + 1) * 128],
                                        idn[:])
                                    nc.vector.tensor_copy(
                                        vn[(NCH - 1) * 4 + t][:], pvt)
                            for g in range(6):
                                emit_S(g)    # bookkeeping only (chain adds)
                            for g in range(NG):
                                emit_PV(g)
                        else:
                            emit_S(0)
                            emit_S(1)
                            for g in range(2, NG):
                                cfill(3 if len(pend) > 60 else (2 if len(pend) > 30 else 1))
                                emit_PV(g - 2)
                                emit_S(g)
                            cfill(1)
                            emit_PV(NG - 2)
                            cfill(1)
                            emit_PV(NG - 1)

                        # sumexp finalize: fold halves, cross-partition
                        # all-reduce on GpSimd, reciprocal, normalize.
                        sh = bx.tile([128, 512], bf16, tag="sh", name="sh",
                                     bufs=3)
                        nc.vector.tensor_tensor(sh[:], acc[:, 0:512],
                                                acc[:, 512:1024], add)
                        sums = bx.tile([128, 512], f32, tag="sums", name="sums",
                                       bufs=3)
                        nc.gpsimd.partition_all_reduce(
                            sums[:], sh[:], channels=128,
                            reduce_op=bass_isa.ReduceOp.add)
                        rec = bx.tile([128, 512], f32, tag="rec", name="rec",
                                      bufs=3)
                        nc.vector.reciprocal(rec[:], sums[:])
                        nc.vector.tensor_tensor(ot[h][qb][:], pso_box["pso"][:],
                                                rec[:], mult)

                    # drain whatever filler remains before the next qb
                    cfill(len(pend))

                # ---- tail: out-proj for the last query block
                pend.extend(make_cwork(NCH - 1, split_store=True))
                cfill(len(pend))

    nc.compile()
    return nc


def _host_inputs(x, Wq, Wk, Wv, Wo):
    import ml_dtypes
    bf16 = ml_dtypes.bfloat16

    inv = 1.0 / (BASE ** (np.arange(0, HD, 2, dtype=np.float32) / HD))
    pos = np.arange(L, dtype=np.float32)
    fr = pos[:, None] * inv[None, :]
    emb = np.concatenate([fr, fr], axis=1)            # [L, HD]
    cosT = np.ascontiguousarray(np.cos(emb).T).astype(bf16)   # [HD, L]
    sinT = np.ascontiguousarray(np.sin(emb).T).astype(bf16)
    idn = np.eye(128, dtype=np.float32)

    # x relayout: xh[p, (n*16+c)*512 + l] = x[b][n*512+l, c*128+p]
    xh = [np.ascontiguousarray(
        x[b].T.reshape(NT, 128, NCH, 512).transpose(1, 2, 0, 3)
        .reshape(128, NCH * NT * 512)).astype(bf16) for b in range(B)]

    maps = []
    for core in range(8):
        b, g = core // 4, core % 4
        WqS = Wq[:, g * NH * HD:(g + 1) * NH * HD]    # [D, 512]
        WkS = Wk[:, g * HD:(g + 1) * HD]              # [D, 128]
        WvS = Wv[:, g * HD:(g + 1) * HD]
        wq_h = np.ascontiguousarray(
            WqS.reshape(NT, 128, 512).transpose(1, 0, 2)
            .reshape(128, NT * 512)).astype(bf16)
        wk_h = np.ascontiguousarray(
            WkS.reshape(NT, 128, 128).transpose(1, 0, 2)
            .reshape(128, NT * 128)).astype(bf16)
        wv_h = np.ascontiguousarray(
            WvS.reshape(NT, 128, 128).transpose(1, 0, 2)
            .reshape(128, NT * 128)).astype(bf16)
        maps.append({
            "xh": xh[b],
            "wq": wq_h, "wk": wk_h, "wv": wv_h,
            "wo": np.ascontiguousarray(
                Wo[g * NH * HD:(g + 1) * NH * HD, :]).astype(bf16),
            "cosT": cosT, "sinT": sinT, "ident": idn,
        })
    return maps


def _run(inputs, trace=False):
    global _compiled
    from concourse.bass_utils import run_bass_kernel_spmd
    if _compiled is None:
        _compiled = _build()
    maps = _host_inputs(inputs["x"], inputs["Wq"], inputs["Wk"],
                        inputs["Wv"], inputs["Wo"])
    res = run_bass_kernel_spmd(_compiled, maps, list(range(8)), trace=trace)
    y = np.empty((B, L, D), np.float32)
    for b in range(B):
        y[b] = res.results[b * 4]["y"]
        for g in range(1, 4):
            y[b] += res.results[b * 4 + g]["y"]
    return y, res


def kernel(**inputs):
    x = np.asarray(inputs["x"], np.float32)
    y, _ = _run({"x": x,
                 "Wq": np.asarray(inputs["Wq"], np.float32),
                 "Wk": np.asarray(inputs["Wk"], np.float32),
                 "Wv": np.asarray(inputs["Wv"], np.float32),
                 "Wo": np.asarray(inputs["Wo"], np.float32)})
    return y
